# revision 19
# baseline (speedup 1.0000x reference)
"""GAT layer (nn_GAT_40037685133531) as a Trainium2 Bass kernel on 8 NeuronCores.

Strategy (graph/data parallel, no collectives):
  - Destination nodes sharded 8 ways (6250 per core); edges assigned to the
    core owning their destination, grouped into 128-destination tiles and
    sorted by src within each tile (for the int16 lo/hi table split).
  - Phase 0 (replicated on every core): htab[n] rows of 512 B hold
    [h(128)|alpha_s(4)] in bf16 (cols 136:256 unwritten); ad_tab[n] holds
    alpha_d. Computed as x @ [W | W@A_src | W@A_dst] in 128-node matmul
    tiles (bf16 in, f32 PSUM, scalar-engine copies).
  - Phase 1 (per destination tile, K = KL+KH slots of 128 edges):
    TWO large dma_gather calls (single_packet=False — this is what makes
    >256-index gathers work) fetch the 512 B rows of all the tile's edges
    by src (lo: src<32768, hi: rest, int16 wrapped indices). alpha_d for
    the tile's 128 own nodes comes from one [P,1] indirect DMA on ad_tab
    and is broadcast to edges with PE transposes of the one-hot S plus
    4-column matmuls. S is built in [P, d, k] layout so the is_equal
    runs in the DVE 2x (16-bit packed) mode. ex = exp(leakyrelu(as+ad)),
    msg = [ex*h | ex] in bf16, and psum += S_j.T @ msg_j accumulates
    weighted feature sums and softmax denominators together.
  - Softmax normalization after aggregation (out = num/denom); segment-max
    subtraction dropped (logits are small; exp is safe). ELU + final
    linear (z @ W2 + b2) per destination tile via a PE transpose.
"""

import os
import sys

import numpy as np

if "/opt/trn_rl_repo" not in sys.path:
    sys.path.insert(0, "/opt/trn_rl_repo")

N_NODES = 50000
N_EDGES = 800000
F_IN = 128
HEADS = 4
HIDDEN = 32
F_OUT = 64
NEG = 0.2
N_CORES = 8
P = 128
FE = F_IN + 2 * HEADS    # 136 phase-0 psum cols: h | alpha_s | alpha_d
FG = F_IN + HEADS        # 132 written htab cols: h | alpha_s
FM = F_IN + HEADS        # 132 message cols: ex*h | ex
EB = 256                 # bf16 elems per htab row (512 B, dma_gather elem)
LO_SPLIT = 32768         # int16 index limit for dma_gather
NODES_PER_CORE = N_NODES // N_CORES          # 6250
T_TILES = (NODES_PER_CORE + P - 1) // P      # 49
NPAD = T_TILES * 8 * P                       # 50176
CH = 8                   # node tiles per phase-0 chunk
JB = 2                   # matmuls per phase-0 PSUM bank (2*136*4B < 2KB)


def _wrap16(idx, n_slots):
    """dma_gather int16 index layout: index i lives at [i % 16, i // 16],
    replicated 8x over partition groups -> [128, 8*n_slots]."""
    full = np.zeros(n_slots * P, np.int16)
    full[:len(idx)] = idx
    return np.tile(full.reshape(n_slots * 8, 16).T, (8, 1))


def _prep(edge_index):
    """CPU-side sharding: group edges by (core, tile) of their destination,
    sort by src within each group, split at the int16 boundary, lay out
    wrapped gather indices and per-slot local-dst arrays (padded to the
    cross-core max slot counts per tile)."""
    import ml_dtypes

    src = np.ascontiguousarray(np.asarray(edge_index[0]).astype(np.int64))
    dst = np.ascontiguousarray(np.asarray(edge_index[1]).astype(np.int64))

    core_of = dst // NODES_PER_CORE
    ltile_of = (dst - core_of * NODES_PER_CORE) // P
    group = core_of * T_TILES + ltile_of
    order = np.lexsort((src, group))
    src_s, dst_s, group_s = src[order], dst[order], group[order]
    lo_mask = src_s < LO_SPLIT

    NG = N_CORES * T_TILES
    gs = np.searchsorted(group_s, np.arange(NG))
    ge = np.searchsorted(group_s, np.arange(NG), side="right")
    n_lo = np.array([np.count_nonzero(lo_mask[gs[g]:ge[g]])
                     for g in range(NG)]).reshape(N_CORES, T_TILES)
    n_hi = (ge - gs).reshape(N_CORES, T_TILES) - n_lo
    KL_list = tuple(int(v) for v in
                    np.ceil(n_lo.max(axis=0) / P).astype(np.int64))
    KH_list = tuple(int(v) for v in
                    np.ceil(n_hi.max(axis=0) / P).astype(np.int64))
    KLmax = max(max(KL_list), 1)
    KHmax = max(max(KH_list), 1)
    Kmax = max(kl + kh for kl, kh in zip(KL_list, KH_list))

    lo_idx = np.zeros((N_CORES, T_TILES, P, 8 * KLmax), np.int16)
    hi_idx = np.zeros((N_CORES, T_TILES, P, 8 * KHmax), np.int16)
    dloc = np.full((N_CORES, T_TILES, P, Kmax), -1.0, np.float32)

    for c in range(N_CORES):
        for t in range(T_TILES):
            g = c * T_TILES + t
            s, e = gs[g], ge[g]
            nl = n_lo[c, t]
            KL = KL_list[t]
            base = c * NODES_PER_CORE + t * P
            if KL:
                lo_idx[c, t, :, :8 * KL] = _wrap16(
                    src_s[s:s + nl].astype(np.int16), KL)
            if KH_list[t]:
                hi_idx[c, t, :, :8 * KH_list[t]] = _wrap16(
                    (src_s[s + nl:e] - LO_SPLIT).astype(np.int16),
                    KH_list[t])
            i_lo = np.arange(nl)
            dloc[c, t, i_lo % P, i_lo // P] = (
                dst_s[s:s + nl] - base).astype(np.float32)
            i_hi = np.arange(e - s - nl)
            dloc[c, t, i_hi % P, KL + i_hi // P] = (
                dst_s[s + nl:e] - base).astype(np.float32)
    return (lo_idx, hi_idx, dloc.astype(ml_dtypes.bfloat16),
            KL_list, KH_list, KLmax, KHmax, Kmax)


def _build_module(KL_list, KH_list, KLmax, KHmax, Kmax, bias_nz, b2_nz):
    import concourse.bass as bass
    import concourse.mybir as mybir
    import concourse.tile as tile
    from concourse import bacc
    from concourse.masks import make_identity

    f32 = mybir.dt.float32
    bf16 = mybir.dt.bfloat16
    i16 = mybir.dt.int16
    i32 = mybir.dt.int32

    nc = bacc.Bacc("TRN2", target_bir_lowering=False, debug=False,
                   num_devices=N_CORES, num_swdge_queues=4)

    x_T = nc.dram_tensor("x_T", [P, NPAD], bf16, kind="ExternalInput")
    W_ext_d = nc.dram_tensor("W_ext", [P, FE], bf16, kind="ExternalInput")
    W2_d = nc.dram_tensor("W2", [P, F_OUT], bf16, kind="ExternalInput")
    lo_d = nc.dram_tensor("lo_idx", [T_TILES, P, 8 * KLmax], i16,
                          kind="ExternalInput")
    hi_d = nc.dram_tensor("hi_idx", [T_TILES, P, 8 * KHmax], i16,
                          kind="ExternalInput")
    dloc_d = nc.dram_tensor("d_local", [T_TILES, P, Kmax], bf16,
                            kind="ExternalInput")
    xloc_d = nc.dram_tensor("x_loc_T", [P, T_TILES * P], bf16,
                            kind="ExternalInput")
    if bias_nz:
        bias_d = nc.dram_tensor("bias_ext", [1, FE], bf16,
                                kind="ExternalInput")
    if b2_nz:
        b2_d = nc.dram_tensor("b2_row", [1, F_OUT], bf16,
                              kind="ExternalInput")
    y_d = nc.dram_tensor("y_out", [T_TILES * P, F_OUT], f32,
                         kind="ExternalOutput")
    htab = nc.dram_tensor("htab", [NPAD, EB], bf16, kind="Internal")

    add = mybir.AluOpType.add
    mult = mybir.AluOpType.mult
    amax = mybir.AluOpType.max
    is_eq = mybir.AluOpType.is_equal
    Exp = mybir.ActivationFunctionType.Exp
    Copy = mybir.ActivationFunctionType.Copy

    with tile.TileContext(nc) as tc:
        with (
            tc.tile_pool(name="const", bufs=1) as constp,
            tc.tile_pool(name="S", bufs=6) as sp,
            tc.tile_pool(name="ST", bufs=4) as stp,
            tc.tile_pool(name="msgs", bufs=3) as mp,
            tc.tile_pool(name="small", bufs=3) as smallp,
        ):
            W_ext_sb = constp.tile([P, FE], bf16)
            nc.sync.dma_start(W_ext_sb[:], W_ext_d.ap())
            W2_sb = constp.tile([P, F_OUT], bf16)
            nc.sync.dma_start(W2_sb[:], W2_d.ap())
            ident = constp.tile([P, P], bf16)
            make_identity(nc, ident[:])
            # iota_dk[p, d, k] = d  (k-minor so the is_equal gets packed
            # 16-bit innermost dims on both operands -> DVE 2x mode)
            iota_d = constp.tile([P, P], bf16)
            nc.gpsimd.iota(iota_d[:], pattern=[[1, P]], base=0,
                           channel_multiplier=0,
                           allow_small_or_imprecise_dtypes=True)
            iota_dk = constp.tile([P, P * Kmax], bf16)
            nc.vector.tensor_copy(
                iota_dk[:].rearrange("p (d k) -> p d k", k=Kmax),
                iota_d[:].unsqueeze(2).to_broadcast([P, P, Kmax]))
            lo_sb = constp.tile([P, T_TILES, 8 * KLmax], i16)
            nc.sync.dma_start(lo_sb[:],
                              lo_d.ap().rearrange("t p k -> p t k"))
            hi_sb = constp.tile([P, T_TILES, 8 * KHmax], i16)
            nc.sync.dma_start(hi_sb[:],
                              hi_d.ap().rearrange("t p k -> p t k"))
            dloc_sb = constp.tile([P, T_TILES, Kmax], bf16)
            nc.sync.dma_start(dloc_sb[:],
                              dloc_d.ap().rearrange("t p k -> p t k"))
            xloc_sb = constp.tile([P, T_TILES * P], bf16)
            nc.sync.dma_start(xloc_sb[:], xloc_d.ap())
            ad_loc = constp.tile([P, T_TILES * HEADS], bf16)
            if bias_nz or b2_nz:
                ones_sb = constp.tile([1, P], bf16)
                nc.vector.memset(ones_sb[:], 1.0)
            if bias_nz:
                bias_sb = constp.tile([1, FE], bf16)
                nc.sync.dma_start(bias_sb[:], bias_d.ap())
            if b2_nz:
                b2_sb = constp.tile([1, F_OUT], bf16)
                nc.sync.dma_start(b2_sb[:], b2_d.ap())

            # ---- phase 0: htab/ad_tab = x @ [W | W@A_s | W@A_d] ----
            with (
                tc.tile_pool(name="xt", bufs=3) as xtp,
                tc.tile_pool(name="hx", bufs=3) as hxp,
                tc.tile_pool(name="p0ps", bufs=6, space="PSUM") as p0ps,
            ):
                for c in range(NPAD // (CH * P)):
                    xt = xtp.tile([P, CH * P], bf16)
                    nc.sync.dma_start(
                        xt[:], x_T.ap()[:, c * CH * P:(c + 1) * CH * P])
                    hx = hxp.tile([P, CH * FE], bf16, tag="hx")
                    for jj in range(CH // JB):
                        ps = p0ps.tile([P, JB * FE], f32)
                        for u in range(JB):
                            j = jj * JB + u
                            nc.tensor.matmul(
                                ps[:, u * FE:(u + 1) * FE],
                                lhsT=xt[:, j * P:(j + 1) * P],
                                rhs=W_ext_sb[:], start=True,
                                stop=not bias_nz)
                            if bias_nz:
                                nc.tensor.matmul(
                                    ps[:, u * FE:(u + 1) * FE],
                                    lhsT=ones_sb[:], rhs=bias_sb[:],
                                    start=False, stop=True)
                        nc.scalar.activation(
                            out=hx[:, jj * JB * FE:(jj + 1) * JB * FE],
                            in_=ps[:], func=Copy)
                    rows = slice(c * CH * P, (c + 1) * CH * P)
                    nc.sync.dma_start(
                        htab.ap()[rows, 0:FE]
                        .rearrange("(t p) e -> p t e", p=P),
                        hx[:].rearrange("p (t e) -> p t e", t=CH))

            # ---- phase 0.5: this core's own-node alpha_d in SBUF ----
            with tc.tile_pool(name="adp0", bufs=2, space="PSUM") as adp0:
                for t in range(T_TILES):
                    ap0 = adp0.tile([P, HEADS], f32)
                    nc.tensor.matmul(ap0[:],
                                     lhsT=xloc_sb[:, t * P:(t + 1) * P],
                                     rhs=W_ext_sb[:, FG:FE],
                                     start=True, stop=True)
                    nc.vector.tensor_copy(
                        ad_loc[:, t * HEADS:(t + 1) * HEADS], ap0[:])

            # ---- phase 1: per destination tile ----
            with (
                tc.tile_pool(name="g", bufs=3) as gp,
                tc.tile_pool(name="agg", bufs=2, space="PSUM") as aggp,
                tc.tile_pool(name="stps", bufs=2, space="PSUM") as stpsp,
                tc.tile_pool(name="adps", bufs=2, space="PSUM") as adpsp,
                tc.tile_pool(name="tr", bufs=1, space="PSUM") as trp,
                tc.tile_pool(name="yps", bufs=1, space="PSUM") as ypsp,
            ):
                qc = 0
                tails = []

                def make_tail(t, ps):
                    def tail():
                        rec = smallp.tile([P, HEADS], f32, tag="rec")
                        nc.vector.tensor_scalar_add(out=rec[:],
                                                    in0=ps[:, F_IN:FM],
                                                    scalar1=1e-16)
                        nc.vector.reciprocal(rec[:], rec[:])
                        zn = smallp.tile([P, F_IN], bf16, tag="zn")
                        nc.vector.tensor_tensor(
                            out=zn[:].rearrange("p (h f) -> p h f", h=HEADS),
                            in0=ps[:, 0:F_IN].rearrange("p (h f) -> p h f",
                                                        h=HEADS),
                            in1=rec[:].unsqueeze(2).to_broadcast(
                                [P, HEADS, HIDDEN]),
                            op=mult)
                        # ELU(z) = max(z, exp(min(z,0)) - 1)
                        tmp = smallp.tile([P, F_IN], bf16, tag="tmp")
                        nc.vector.tensor_scalar_min(out=tmp[:], in0=zn[:],
                                                    scalar1=0.0)
                        nc.scalar.activation(out=tmp[:], in_=tmp[:],
                                             func=Exp)
                        zel = smallp.tile([P, F_IN], bf16, tag="zel")
                        nc.vector.scalar_tensor_tensor(
                            out=zel[:], in0=tmp[:], scalar=-1.0, in1=zn[:],
                            op0=add, op1=amax)

                        pt = trp.tile([P, P], bf16, tag="pt")
                        nc.tensor.transpose(out=pt[:], in_=zel[:],
                                            identity=ident[:])
                        znT = smallp.tile([P, P], bf16, tag="znT")
                        nc.scalar.activation(out=znT[:], in_=pt[:],
                                             func=Copy)
                        yp = ypsp.tile([P, F_OUT], f32, tag="yp")
                        nc.tensor.matmul(yp[:], lhsT=znT[:], rhs=W2_sb[:],
                                         start=True, stop=not b2_nz)
                        if b2_nz:
                            nc.tensor.matmul(yp[:], lhsT=ones_sb[:],
                                             rhs=b2_sb[:],
                                             start=False, stop=True)
                        ysb = smallp.tile([P, F_OUT], f32, tag="ysb")
                        nc.scalar.activation(out=ysb[:], in_=yp[:],
                                             func=Copy)
                        nc.sync.dma_start(y_d.ap()[t * P:(t + 1) * P, :],
                                          ysb[:])
                    return tail

                for t in range(T_TILES):
                    KL, KH = KL_list[t], KH_list[t]
                    K = KL + KH
                    g = gp.tile([P, K * EB], bf16)
                    g3 = g[:].rearrange("p (k e) -> p k e", k=K)
                    for a, b in ((0, KL // 2), (KL // 2, KL)):
                        if b > a:
                            nc.gpsimd.dma_gather(
                                out_ap=g3[:, a:b, :], in_ap=htab.ap(),
                                idxs_ap=lo_sb[:, t, 8 * a:8 * b],
                                num_idxs=(b - a) * P,
                                num_idxs_reg=(b - a) * P,
                                elem_size=EB, single_packet=False,
                                queue_num=qc % 4)
                            qc += 1
                    for a, b in ((0, KH // 2), (KH // 2, KH)):
                        if b > a:
                            nc.gpsimd.dma_gather(
                                out_ap=g3[:, KL + a:KL + b, :],
                                in_ap=htab.ap()[LO_SPLIT:, :],
                                idxs_ap=hi_sb[:, t, 8 * a:8 * b],
                                num_idxs=(b - a) * P,
                                num_idxs_reg=(b - a) * P,
                                elem_size=EB, single_packet=False,
                                queue_num=qc % 4)
                            qc += 1

                    # one-hot scatter matrix S[p=e, (d, k)] (k-minor)
                    S = sp.tile([P, P * K], bf16)
                    S3 = S[:].rearrange("p (d k) -> p d k", k=K)
                    nc.vector.tensor_tensor(
                        out=S3,
                        in0=dloc_sb[:, t, 0:K].unsqueeze(1)
                        .to_broadcast([P, P, K]),
                        in1=iota_dk[:].rearrange(
                            "p (d k) -> p d k", k=Kmax)[:, :, 0:K],
                        op=is_eq)

                    # ST_j = S_j^T via PE transposes, 8 per PSUM bank
                    ST = stp.tile([P, K * P], bf16)
                    for b in range((K + 7) // 8):
                        j0 = b * 8
                        j1 = min(K, j0 + 8)
                        stps = stpsp.tile([P, 1024], bf16, tag="stps")
                        for j in range(j0, j1):
                            nc.tensor.transpose(
                                out=stps[:, (j - j0) * P:(j - j0 + 1) * P],
                                in_=S3[:, :, j], identity=ident[:])
                        nc.scalar.activation(
                            out=ST[:, j0 * P:j1 * P],
                            in_=stps[:, 0:(j1 - j0) * P], func=Copy)
                    # alpha_d per edge: [P, K*4] = ST_j.T @ adt per slot
                    adps = adpsp.tile([P, K * HEADS], f32)
                    for j in range(K):
                        nc.tensor.matmul(
                            adps[:, j * HEADS:(j + 1) * HEADS],
                            lhsT=ST[:, j * P:(j + 1) * P],
                            rhs=ad_loc[:, t * HEADS:(t + 1) * HEADS],
                            start=True, stop=True)

                    msgs = mp.tile([P, K * FM], bf16)
                    m3 = msgs[:].rearrange("p (k f) -> p k f", k=K)
                    ex = m3[:, :, F_IN:FM]                       # [P, K, 4]
                    nc.vector.tensor_tensor(
                        out=ex, in0=g3[:, :, F_IN:F_IN + HEADS],
                        in1=adps[:].rearrange("p (k h) -> p k h", k=K),
                        op=add)
                    nc.vector.scalar_tensor_tensor(
                        out=ex, in0=ex, scalar=NEG, in1=ex,
                        op0=mult, op1=amax)
                    nc.scalar.activation(out=ex, in_=ex, func=Exp)
                    nc.vector.tensor_tensor(
                        out=m3[:, :, 0:F_IN].rearrange(
                            "p k (h f) -> p k h f", h=HEADS),
                        in0=g3[:, :, 0:F_IN].rearrange(
                            "p k (h f) -> p k h f", h=HEADS),
                        in1=ex.unsqueeze(3).to_broadcast(
                            [P, K, HEADS, HIDDEN]),
                        op=mult)

                    ps = aggp.tile([P, FM], f32)
                    for j in range(K):
                        nc.tensor.matmul(ps[:], lhsT=S3[:, :, j],
                                         rhs=msgs[:, j * FM:(j + 1) * FM],
                                         start=(j == 0), stop=(j == K - 1))

                    make_tail(t, ps)()

    nc.compile()
    return nc


_MODULE_CACHE = {}


def _get_module(KL_list, KH_list, KLmax, KHmax, Kmax, bias_nz, b2_nz):
    key = (KL_list, KH_list, bias_nz, b2_nz)
    if key not in _MODULE_CACHE:
        _MODULE_CACHE[key] = _build_module(KL_list, KH_list, KLmax, KHmax,
                                           Kmax, bias_nz, b2_nz)
    return _MODULE_CACHE[key]


def _make_inputs(x, edge_index, edge_weight, W, a_src, a_dst, bias, W2, b2):
    """Shared CPU prep: returns (nc, in_maps)."""
    import ml_dtypes
    bf = ml_dtypes.bfloat16

    x = np.ascontiguousarray(np.asarray(x, np.float32))
    W = np.asarray(W, np.float32)
    a_src = np.asarray(a_src, np.float32)
    a_dst = np.asarray(a_dst, np.float32)
    bias = np.asarray(bias, np.float32)
    W2 = np.ascontiguousarray(np.asarray(W2, np.float32))
    b2 = np.asarray(b2, np.float32)

    A_s = np.zeros((F_IN, HEADS), np.float32)
    A_d = np.zeros((F_IN, HEADS), np.float32)
    for h in range(HEADS):
        A_s[h * HIDDEN:(h + 1) * HIDDEN, h] = a_src[h]
        A_d[h * HIDDEN:(h + 1) * HIDDEN, h] = a_dst[h]
    W_ext = np.concatenate([W, W @ A_s, W @ A_d], axis=1).astype(bf)

    (lo_idx, hi_idx, dloc, KL_list, KH_list, KLmax, KHmax,
     Kmax) = _prep(edge_index)

    bias_nz = bool(np.any(bias))
    b2_nz = bool(np.any(b2))
    nc = _get_module(KL_list, KH_list, KLmax, KHmax, Kmax, bias_nz, b2_nz)

    x_T = np.zeros((P, NPAD), bf)
    x_T[:, :N_NODES] = x.T.astype(bf)
    W2_b = W2.astype(bf)
    x_pad = np.zeros((NPAD, F_IN), np.float32)
    x_pad[:N_NODES] = x

    in_maps = []
    for c in range(N_CORES):
        m = {
            "x_T": x_T,
            "W_ext": W_ext,
            "W2": W2_b,
            "lo_idx": np.ascontiguousarray(lo_idx[c]),
            "hi_idx": np.ascontiguousarray(hi_idx[c]),
            "d_local": np.ascontiguousarray(dloc[c]),
            "x_loc_T": np.ascontiguousarray(
                x_pad[c * NODES_PER_CORE:
                      c * NODES_PER_CORE + T_TILES * P].T.astype(bf)),
        }
        if bias_nz:
            be = np.zeros((1, FE), np.float32)
            be[0, :F_IN] = bias
            m["bias_ext"] = be.astype(bf)
        if b2_nz:
            m["b2_row"] = b2.reshape(1, F_OUT).astype(bf)
        in_maps.append(m)
    return nc, in_maps


def _ensure_ntff_hook():
    """The axon NTFF profile hook lives in antenv.axon_hooks, which this
    image's antenv package lacks; shim it so trace=True works."""
    try:
        import antenv.axon_hooks  # noqa: F401
        return
    except ImportError:
        pass
    import types

    import antenv

    mod = types.ModuleType("antenv.axon_hooks")
    holder = {"h": None}
    mod.set_axon_ntff_profile_hook = lambda h: holder.__setitem__("h", h)
    mod.get_axon_ntff_profile_hook = lambda: holder["h"]
    try:
        from trn_agent_boot.trn_boot import _ntff_profile_via_ctypes
        holder["h"] = _ntff_profile_via_ctypes("/opt/axon/libaxon_pjrt.so")
    except Exception:
        pass
    sys.modules["antenv.axon_hooks"] = mod
    antenv.axon_hooks = mod


def kernel(x, edge_index, edge_weight, W, a_src, a_dst, bias, W2, b2,
           _trace=False):
    from concourse.bass_utils import run_bass_kernel_spmd

    if _trace:
        _ensure_ntff_hook()

    nc, in_maps = _make_inputs(x, edge_index, edge_weight, W, a_src, a_dst,
                               bias, W2, b2)

    res = run_bass_kernel_spmd(nc, in_maps, core_ids=list(range(N_CORES)),
                               trace=_trace)
    out = np.concatenate(
        [res.results[c]["y_out"][:NODES_PER_CORE] for c in range(N_CORES)],
        axis=0)
    if _trace:
        kernel.last_results = res
    return out


# revision 20
# speedup vs baseline: 1.2567x; 1.2567x over previous
"""GAT layer (nn_GAT_40037685133531) as a Trainium2 Bass kernel on 8 NeuronCores.

Strategy (graph/data parallel, no collectives):
  - Destination nodes sharded 8 ways (6250 per core); edges assigned to the
    core owning their destination, grouped into 128-destination tiles and
    sorted by src within each tile (for the int16 lo/hi table split).
  - Phase 0 (replicated on every core): htab[n] rows of 512 B hold
    [h(128)|alpha_s(4)] in bf16 (cols 136:256 unwritten); ad_tab[n] holds
    alpha_d. Computed as x @ [W | W@A_src | W@A_dst] in 128-node matmul
    tiles (bf16 in, f32 PSUM, scalar-engine copies).
  - Phase 1 (per destination tile, K = KL+KH slots of 128 edges):
    TWO large dma_gather calls (single_packet=False — this is what makes
    >256-index gathers work) fetch the 512 B rows of all the tile's edges
    by src (lo: src<32768, hi: rest, int16 wrapped indices). alpha_d for
    the tile's 128 own nodes comes from one [P,1] indirect DMA on ad_tab
    and is broadcast to edges with PE transposes of the one-hot S plus
    4-column matmuls. S is built in [P, d, k] layout so the is_equal
    runs in the DVE 2x (16-bit packed) mode. ex = exp(leakyrelu(as+ad)),
    msg = [ex*h | ex] in bf16, and psum += S_j.T @ msg_j accumulates
    weighted feature sums and softmax denominators together.
  - Softmax normalization after aggregation (out = num/denom); segment-max
    subtraction dropped (logits are small; exp is safe). ELU + final
    linear (z @ W2 + b2) per destination tile via a PE transpose.
"""

import os
import sys

import numpy as np

if "/opt/trn_rl_repo" not in sys.path:
    sys.path.insert(0, "/opt/trn_rl_repo")

N_NODES = 50000
N_EDGES = 800000
F_IN = 128
HEADS = 4
HIDDEN = 32
F_OUT = 64
NEG = 0.2
N_CORES = 8
P = 128
FE = F_IN + 2 * HEADS    # 136 phase-0 psum cols: h | alpha_s | alpha_d
FG = F_IN + HEADS        # 132 written htab cols: h | alpha_s
FM = F_IN + HEADS        # 132 message cols: ex*h | ex
EB = 256                 # bf16 elems per htab row (512 B, dma_gather elem)
LO_SPLIT = 32768         # int16 index limit for dma_gather
NODES_PER_CORE = N_NODES // N_CORES          # 6250
T_TILES = (NODES_PER_CORE + P - 1) // P      # 49
NPAD = T_TILES * 8 * P                       # 50176
CH = 8                   # node tiles per phase-0 chunk
JB = 2                   # matmuls per phase-0 PSUM bank (2*136*4B < 2KB)


def _wrap16(idx, n_slots):
    """dma_gather int16 index layout: index i lives at [i % 16, i // 16],
    replicated 8x over partition groups -> [128, 8*n_slots]."""
    full = np.zeros(n_slots * P, np.int16)
    full[:len(idx)] = idx
    return np.tile(full.reshape(n_slots * 8, 16).T, (8, 1))


def _prep(edge_index):
    """CPU-side sharding: group edges by (core, tile) of their destination,
    sort by src within each group, split at the int16 boundary, lay out
    wrapped gather indices and per-slot local-dst arrays (padded to the
    cross-core max slot counts per tile)."""
    import ml_dtypes

    src = np.ascontiguousarray(np.asarray(edge_index[0]).astype(np.int64))
    dst = np.ascontiguousarray(np.asarray(edge_index[1]).astype(np.int64))

    core_of = dst // NODES_PER_CORE
    ltile_of = (dst - core_of * NODES_PER_CORE) // P
    group = core_of * T_TILES + ltile_of
    order = np.lexsort((src, group))
    src_s, dst_s, group_s = src[order], dst[order], group[order]
    lo_mask = src_s < LO_SPLIT

    NG = N_CORES * T_TILES
    gs = np.searchsorted(group_s, np.arange(NG))
    ge = np.searchsorted(group_s, np.arange(NG), side="right")
    n_lo = np.array([np.count_nonzero(lo_mask[gs[g]:ge[g]])
                     for g in range(NG)]).reshape(N_CORES, T_TILES)
    n_hi = (ge - gs).reshape(N_CORES, T_TILES) - n_lo
    KL_list = tuple(int(v) for v in
                    np.ceil(n_lo.max(axis=0) / P).astype(np.int64))
    KH_list = tuple(int(v) for v in
                    np.ceil(n_hi.max(axis=0) / P).astype(np.int64))
    KLmax = max(max(KL_list), 1)
    KHmax = max(max(KH_list), 1)
    Kmax = max(kl + kh for kl, kh in zip(KL_list, KH_list))

    lo_idx = np.zeros((N_CORES, T_TILES, P, 8 * KLmax), np.int16)
    hi_idx = np.zeros((N_CORES, T_TILES, P, 8 * KHmax), np.int16)
    dloc = np.full((N_CORES, T_TILES, P, Kmax), -1.0, np.float32)

    for c in range(N_CORES):
        for t in range(T_TILES):
            g = c * T_TILES + t
            s, e = gs[g], ge[g]
            nl = n_lo[c, t]
            KL = KL_list[t]
            base = c * NODES_PER_CORE + t * P
            if KL:
                lo_idx[c, t, :, :8 * KL] = _wrap16(
                    src_s[s:s + nl].astype(np.int16), KL)
            if KH_list[t]:
                hi_idx[c, t, :, :8 * KH_list[t]] = _wrap16(
                    (src_s[s + nl:e] - LO_SPLIT).astype(np.int16),
                    KH_list[t])
            i_lo = np.arange(nl)
            dloc[c, t, i_lo % P, i_lo // P] = (
                dst_s[s:s + nl] - base).astype(np.float32)
            i_hi = np.arange(e - s - nl)
            dloc[c, t, i_hi % P, KL + i_hi // P] = (
                dst_s[s + nl:e] - base).astype(np.float32)
    return (lo_idx, hi_idx, dloc.astype(ml_dtypes.bfloat16),
            KL_list, KH_list, KLmax, KHmax, Kmax)


def _build_module(KL_list, KH_list, KLmax, KHmax, Kmax, bias_nz, b2_nz):
    import concourse.bass as bass
    import concourse.mybir as mybir
    import concourse.tile as tile
    from concourse import bacc
    from concourse.masks import make_identity

    f32 = mybir.dt.float32
    bf16 = mybir.dt.bfloat16
    i16 = mybir.dt.int16
    i32 = mybir.dt.int32

    nc = bacc.Bacc("TRN2", target_bir_lowering=False, debug=False,
                   num_devices=N_CORES, num_swdge_queues=4)

    x_T = nc.dram_tensor("x_T", [P, NPAD], bf16, kind="ExternalInput")
    W_ext_d = nc.dram_tensor("W_ext", [P, FE], bf16, kind="ExternalInput")
    W2_d = nc.dram_tensor("W2", [P, F_OUT], bf16, kind="ExternalInput")
    lo_d = nc.dram_tensor("lo_idx", [T_TILES, P, 8 * KLmax], i16,
                          kind="ExternalInput")
    hi_d = nc.dram_tensor("hi_idx", [T_TILES, P, 8 * KHmax], i16,
                          kind="ExternalInput")
    dloc_d = nc.dram_tensor("d_local", [T_TILES, P, Kmax], bf16,
                            kind="ExternalInput")
    xloc_d = nc.dram_tensor("x_loc_T", [P, T_TILES * P], bf16,
                            kind="ExternalInput")
    if bias_nz:
        bias_d = nc.dram_tensor("bias_ext", [1, FE], bf16,
                                kind="ExternalInput")
    if b2_nz:
        b2_d = nc.dram_tensor("b2_row", [1, F_OUT], bf16,
                              kind="ExternalInput")
    y_d = nc.dram_tensor("y_out", [T_TILES * P, F_OUT], f32,
                         kind="ExternalOutput")
    htab = nc.dram_tensor("htab", [NPAD, EB], bf16, kind="Internal")

    add = mybir.AluOpType.add
    mult = mybir.AluOpType.mult
    amax = mybir.AluOpType.max
    is_eq = mybir.AluOpType.is_equal
    Exp = mybir.ActivationFunctionType.Exp
    Copy = mybir.ActivationFunctionType.Copy

    with tile.TileContext(nc) as tc:
        with tc.tile_pool(name="const", bufs=1) as constp:
            W_ext_sb = constp.tile([P, FE], bf16)
            nc.sync.dma_start(W_ext_sb[:], W_ext_d.ap())
            W2_sb = constp.tile([P, F_OUT], bf16)
            nc.sync.dma_start(W2_sb[:], W2_d.ap())
            ident = constp.tile([P, P], bf16)
            make_identity(nc, ident[:])
            # iota_dk[p, d, k] = d  (k-minor so the is_equal gets packed
            # 16-bit innermost dims on both operands -> DVE 2x mode)
            iota_d = constp.tile([P, P], bf16)
            nc.gpsimd.iota(iota_d[:], pattern=[[1, P]], base=0,
                           channel_multiplier=0,
                           allow_small_or_imprecise_dtypes=True)
            iota_dk = constp.tile([P, P * Kmax], bf16)
            nc.vector.tensor_copy(
                iota_dk[:].rearrange("p (d k) -> p d k", k=Kmax),
                iota_d[:].unsqueeze(2).to_broadcast([P, P, Kmax]))
            lo_sb = constp.tile([P, T_TILES, 8 * KLmax], i16)
            nc.sync.dma_start(lo_sb[:],
                              lo_d.ap().rearrange("t p k -> p t k"))
            hi_sb = constp.tile([P, T_TILES, 8 * KHmax], i16)
            nc.sync.dma_start(hi_sb[:],
                              hi_d.ap().rearrange("t p k -> p t k"))
            dloc_sb = constp.tile([P, T_TILES, Kmax], bf16)
            nc.sync.dma_start(dloc_sb[:],
                              dloc_d.ap().rearrange("t p k -> p t k"))
            xloc_sb = constp.tile([P, T_TILES * P], bf16)
            nc.sync.dma_start(xloc_sb[:], xloc_d.ap())
            ad_loc = constp.tile([P, T_TILES * HEADS], bf16)
            if bias_nz or b2_nz:
                ones_sb = constp.tile([1, P], bf16)
                nc.vector.memset(ones_sb[:], 1.0)
            if bias_nz:
                bias_sb = constp.tile([1, FE], bf16)
                nc.sync.dma_start(bias_sb[:], bias_d.ap())
            if b2_nz:
                b2_sb = constp.tile([1, F_OUT], bf16)
                nc.sync.dma_start(b2_sb[:], b2_d.ap())

            # ---- phase 0: htab/ad_tab = x @ [W | W@A_s | W@A_d] ----
            with (
                tc.tile_pool(name="xt", bufs=3) as xtp,
                tc.tile_pool(name="hx", bufs=3) as hxp,
                tc.tile_pool(name="p0ps", bufs=6, space="PSUM") as p0ps,
            ):
                for c in range(NPAD // (CH * P)):
                    xt = xtp.tile([P, CH * P], bf16)
                    nc.sync.dma_start(
                        xt[:], x_T.ap()[:, c * CH * P:(c + 1) * CH * P])
                    hx = hxp.tile([P, CH * FG], bf16, tag="hx")
                    for jj in range(CH // JB):
                        ps = p0ps.tile([P, JB * FE], f32)
                        for u in range(JB):
                            j = jj * JB + u
                            nc.tensor.matmul(
                                ps[:, u * FE:(u + 1) * FE],
                                lhsT=xt[:, j * P:(j + 1) * P],
                                rhs=W_ext_sb[:], start=True,
                                stop=not bias_nz)
                            if bias_nz:
                                nc.tensor.matmul(
                                    ps[:, u * FE:(u + 1) * FE],
                                    lhsT=ones_sb[:], rhs=bias_sb[:],
                                    start=False, stop=True)
                        ps3 = ps[:].rearrange("p (u e) -> p u e", u=JB)
                        nc.scalar.activation(
                            out=hx[:, jj * JB * FG:(jj + 1) * JB * FG]
                            .rearrange("p (u e) -> p u e", u=JB),
                            in_=ps3[:, :, 0:FG], func=Copy)
                    rows = slice(c * CH * P, (c + 1) * CH * P)
                    nc.sync.dma_start(
                        htab.ap()[rows, 0:FG]
                        .rearrange("(t p) e -> p t e", p=P),
                        hx[:].rearrange("p (t e) -> p t e", t=CH))

            # ---- phase 0.5: this core's own-node alpha_d in SBUF ----
            with tc.tile_pool(name="adp0", bufs=2, space="PSUM") as adp0:
                for t in range(T_TILES):
                    ap0 = adp0.tile([P, HEADS], f32)
                    nc.tensor.matmul(ap0[:],
                                     lhsT=xloc_sb[:, t * P:(t + 1) * P],
                                     rhs=W_ext_sb[:, FG:FE],
                                     start=True, stop=True)
                    nc.vector.tensor_copy(
                        ad_loc[:, t * HEADS:(t + 1) * HEADS], ap0[:])

            # ---- phase 1: per destination tile ----
            with (
                tc.tile_pool(name="g", bufs=3) as gp,
                tc.tile_pool(name="S", bufs=3) as sp,
                tc.tile_pool(name="ST", bufs=3) as stp,
                tc.tile_pool(name="msgs", bufs=3) as mp,
                tc.tile_pool(name="small", bufs=3) as smallp,
                tc.tile_pool(name="agg", bufs=2, space="PSUM") as aggp,
                tc.tile_pool(name="stps", bufs=2, space="PSUM") as stpsp,
                tc.tile_pool(name="adps", bufs=2, space="PSUM") as adpsp,
                tc.tile_pool(name="tr", bufs=1, space="PSUM") as trp,
                tc.tile_pool(name="yps", bufs=1, space="PSUM") as ypsp,
            ):
                qc = 0
                tails = []

                def make_tail(t, ps):
                    def tail():
                        rec = smallp.tile([P, HEADS], f32, tag="rec")
                        nc.vector.tensor_scalar_add(out=rec[:],
                                                    in0=ps[:, F_IN:FM],
                                                    scalar1=1e-16)
                        nc.vector.reciprocal(rec[:], rec[:])
                        zn = smallp.tile([P, F_IN], bf16, tag="zn")
                        nc.vector.tensor_tensor(
                            out=zn[:].rearrange("p (h f) -> p h f", h=HEADS),
                            in0=ps[:, 0:F_IN].rearrange("p (h f) -> p h f",
                                                        h=HEADS),
                            in1=rec[:].unsqueeze(2).to_broadcast(
                                [P, HEADS, HIDDEN]),
                            op=mult)
                        # ELU(z) = max(z, exp(min(z,0)) - 1)
                        tmp = smallp.tile([P, F_IN], bf16, tag="tmp")
                        nc.vector.tensor_scalar_min(out=tmp[:], in0=zn[:],
                                                    scalar1=0.0)
                        nc.scalar.activation(out=tmp[:], in_=tmp[:],
                                             func=Exp)
                        zel = smallp.tile([P, F_IN], bf16, tag="zel")
                        nc.vector.scalar_tensor_tensor(
                            out=zel[:], in0=tmp[:], scalar=-1.0, in1=zn[:],
                            op0=add, op1=amax)

                        pt = trp.tile([P, P], bf16, tag="pt")
                        nc.tensor.transpose(out=pt[:], in_=zel[:],
                                            identity=ident[:])
                        znT = smallp.tile([P, P], bf16, tag="znT")
                        nc.scalar.activation(out=znT[:], in_=pt[:],
                                             func=Copy)
                        yp = ypsp.tile([P, F_OUT], f32, tag="yp")
                        nc.tensor.matmul(yp[:], lhsT=znT[:], rhs=W2_sb[:],
                                         start=True, stop=not b2_nz)
                        if b2_nz:
                            nc.tensor.matmul(yp[:], lhsT=ones_sb[:],
                                             rhs=b2_sb[:],
                                             start=False, stop=True)
                        ysb = smallp.tile([P, F_OUT], f32, tag="ysb")
                        nc.scalar.activation(out=ysb[:], in_=yp[:],
                                             func=Copy)
                        nc.sync.dma_start(y_d.ap()[t * P:(t + 1) * P, :],
                                          ysb[:])
                    return tail

                for t in range(T_TILES):
                    KL, KH = KL_list[t], KH_list[t]
                    K = KL + KH
                    g = gp.tile([P, K * EB], bf16)
                    g3 = g[:].rearrange("p (k e) -> p k e", k=K)
                    for a, b in ((0, KL // 2), (KL // 2, KL)):
                        if b > a:
                            nc.gpsimd.dma_gather(
                                out_ap=g3[:, a:b, :], in_ap=htab.ap(),
                                idxs_ap=lo_sb[:, t, 8 * a:8 * b],
                                num_idxs=(b - a) * P,
                                num_idxs_reg=(b - a) * P,
                                elem_size=EB, single_packet=False,
                                queue_num=qc % 4)
                            qc += 1
                    for a, b in ((0, KH // 2), (KH // 2, KH)):
                        if b > a:
                            nc.gpsimd.dma_gather(
                                out_ap=g3[:, KL + a:KL + b, :],
                                in_ap=htab.ap()[LO_SPLIT:, :],
                                idxs_ap=hi_sb[:, t, 8 * a:8 * b],
                                num_idxs=(b - a) * P,
                                num_idxs_reg=(b - a) * P,
                                elem_size=EB, single_packet=False,
                                queue_num=qc % 4)
                            qc += 1

                    # one-hot scatter matrix S[p=e, (d, k)] (k-minor)
                    S = sp.tile([P, P * K], bf16)
                    S3 = S[:].rearrange("p (d k) -> p d k", k=K)
                    nc.vector.tensor_tensor(
                        out=S3,
                        in0=dloc_sb[:, t, 0:K].unsqueeze(1)
                        .to_broadcast([P, P, K]),
                        in1=iota_dk[:].rearrange(
                            "p (d k) -> p d k", k=Kmax)[:, :, 0:K],
                        op=is_eq)

                    # ST_j = S_j^T via PE transposes, 8 per PSUM bank
                    ST = stp.tile([P, K * P], bf16)
                    for b in range((K + 7) // 8):
                        j0 = b * 8
                        j1 = min(K, j0 + 8)
                        stps = stpsp.tile([P, 1024], bf16, tag="stps")
                        for j in range(j0, j1):
                            nc.tensor.transpose(
                                out=stps[:, (j - j0) * P:(j - j0 + 1) * P],
                                in_=S3[:, :, j], identity=ident[:])
                        nc.scalar.activation(
                            out=ST[:, j0 * P:j1 * P],
                            in_=stps[:, 0:(j1 - j0) * P], func=Copy)
                    # alpha_d per edge: [P, K*4] = ST_j.T @ adt per slot
                    adps = adpsp.tile([P, K * HEADS], f32)
                    for j in range(K):
                        nc.tensor.matmul(
                            adps[:, j * HEADS:(j + 1) * HEADS],
                            lhsT=ST[:, j * P:(j + 1) * P],
                            rhs=ad_loc[:, t * HEADS:(t + 1) * HEADS],
                            start=True, stop=True)

                    msgs = mp.tile([P, K * FM], bf16)
                    m3 = msgs[:].rearrange("p (k f) -> p k f", k=K)
                    ex = m3[:, :, F_IN:FM]                       # [P, K, 4]
                    nc.vector.tensor_tensor(
                        out=ex, in0=g3[:, :, F_IN:F_IN + HEADS],
                        in1=adps[:].rearrange("p (k h) -> p k h", k=K),
                        op=add)
                    nc.vector.scalar_tensor_tensor(
                        out=ex, in0=ex, scalar=NEG, in1=ex,
                        op0=mult, op1=amax)
                    nc.scalar.activation(out=ex, in_=ex, func=Exp)
                    nc.vector.tensor_tensor(
                        out=m3[:, :, 0:F_IN].rearrange(
                            "p k (h f) -> p k h f", h=HEADS),
                        in0=g3[:, :, 0:F_IN].rearrange(
                            "p k (h f) -> p k h f", h=HEADS),
                        in1=ex.unsqueeze(3).to_broadcast(
                            [P, K, HEADS, HIDDEN]),
                        op=mult)

                    ps = aggp.tile([P, FM], f32)
                    for j in range(K):
                        nc.tensor.matmul(ps[:], lhsT=S3[:, :, j],
                                         rhs=msgs[:, j * FM:(j + 1) * FM],
                                         start=(j == 0), stop=(j == K - 1))

                    make_tail(t, ps)()

    nc.compile()
    return nc


_MODULE_CACHE = {}


def _get_module(KL_list, KH_list, KLmax, KHmax, Kmax, bias_nz, b2_nz):
    key = (KL_list, KH_list, bias_nz, b2_nz)
    if key not in _MODULE_CACHE:
        _MODULE_CACHE[key] = _build_module(KL_list, KH_list, KLmax, KHmax,
                                           Kmax, bias_nz, b2_nz)
    return _MODULE_CACHE[key]


def _make_inputs(x, edge_index, edge_weight, W, a_src, a_dst, bias, W2, b2):
    """Shared CPU prep: returns (nc, in_maps)."""
    import ml_dtypes
    bf = ml_dtypes.bfloat16

    x = np.ascontiguousarray(np.asarray(x, np.float32))
    W = np.asarray(W, np.float32)
    a_src = np.asarray(a_src, np.float32)
    a_dst = np.asarray(a_dst, np.float32)
    bias = np.asarray(bias, np.float32)
    W2 = np.ascontiguousarray(np.asarray(W2, np.float32))
    b2 = np.asarray(b2, np.float32)

    A_s = np.zeros((F_IN, HEADS), np.float32)
    A_d = np.zeros((F_IN, HEADS), np.float32)
    for h in range(HEADS):
        A_s[h * HIDDEN:(h + 1) * HIDDEN, h] = a_src[h]
        A_d[h * HIDDEN:(h + 1) * HIDDEN, h] = a_dst[h]
    W_ext = np.concatenate([W, W @ A_s, W @ A_d], axis=1).astype(bf)

    (lo_idx, hi_idx, dloc, KL_list, KH_list, KLmax, KHmax,
     Kmax) = _prep(edge_index)

    bias_nz = bool(np.any(bias))
    b2_nz = bool(np.any(b2))
    nc = _get_module(KL_list, KH_list, KLmax, KHmax, Kmax, bias_nz, b2_nz)

    x_T = np.zeros((P, NPAD), bf)
    x_T[:, :N_NODES] = x.T.astype(bf)
    W2_b = W2.astype(bf)
    x_pad = np.zeros((NPAD, F_IN), np.float32)
    x_pad[:N_NODES] = x

    in_maps = []
    for c in range(N_CORES):
        m = {
            "x_T": x_T,
            "W_ext": W_ext,
            "W2": W2_b,
            "lo_idx": np.ascontiguousarray(lo_idx[c]),
            "hi_idx": np.ascontiguousarray(hi_idx[c]),
            "d_local": np.ascontiguousarray(dloc[c]),
            "x_loc_T": np.ascontiguousarray(
                x_pad[c * NODES_PER_CORE:
                      c * NODES_PER_CORE + T_TILES * P].T.astype(bf)),
        }
        if bias_nz:
            be = np.zeros((1, FE), np.float32)
            be[0, :F_IN] = bias
            m["bias_ext"] = be.astype(bf)
        if b2_nz:
            m["b2_row"] = b2.reshape(1, F_OUT).astype(bf)
        in_maps.append(m)
    return nc, in_maps


def _ensure_ntff_hook():
    """The axon NTFF profile hook lives in antenv.axon_hooks, which this
    image's antenv package lacks; shim it so trace=True works."""
    try:
        import antenv.axon_hooks  # noqa: F401
        return
    except ImportError:
        pass
    import types

    import antenv

    mod = types.ModuleType("antenv.axon_hooks")
    holder = {"h": None}
    mod.set_axon_ntff_profile_hook = lambda h: holder.__setitem__("h", h)
    mod.get_axon_ntff_profile_hook = lambda: holder["h"]
    try:
        from trn_agent_boot.trn_boot import _ntff_profile_via_ctypes
        holder["h"] = _ntff_profile_via_ctypes("/opt/axon/libaxon_pjrt.so")
    except Exception:
        pass
    sys.modules["antenv.axon_hooks"] = mod
    antenv.axon_hooks = mod


def kernel(x, edge_index, edge_weight, W, a_src, a_dst, bias, W2, b2,
           _trace=False):
    from concourse.bass_utils import run_bass_kernel_spmd

    if _trace:
        _ensure_ntff_hook()

    nc, in_maps = _make_inputs(x, edge_index, edge_weight, W, a_src, a_dst,
                               bias, W2, b2)

    res = run_bass_kernel_spmd(nc, in_maps, core_ids=list(range(N_CORES)),
                               trace=_trace)
    out = np.concatenate(
        [res.results[c]["y_out"][:NODES_PER_CORE] for c in range(N_CORES)],
        axis=0)
    if _trace:
        kernel.last_results = res
    return out
